# revision 8
# baseline (speedup 1.0000x reference)
"""AssocScan Trainium2 kernel: out[:, t] = gates[:, t] * out[:, t-1] + inputs[:, t].

Strategy: the recurrence is independent per (b, d) lane (B*D = 4096 lanes,
N = 4096 steps). The DVE `tensor_tensor_scan` instruction computes exactly
this recurrence along the free dimension, one lane per partition.

Sharding: lanes are split evenly across the 8 NeuronCores (512 lanes each).
During host-side sharding the (B, N, D) inputs are transposed to lane-major
(B*D, N) so every device DMA is fully contiguous (time series per lane
contiguous in DRAM); each core loads its shard with two 8 MiB DMAs, scans
4 tiles of [128 lanes, 4096 steps] in place on the VectorEngine, and
streams the results back.
"""

import sys

import numpy as np

for _p in ("/opt/trn_rl_repo", "/opt/pypackages"):
    if _p not in sys.path:
        sys.path.append(_p)

import concourse.bacc as bacc
import concourse.mybir as mybir
from concourse.bass_utils import run_bass_kernel_spmd
from concourse.tile import TileContext

B, N, D = 4, 4096, 1024
N_CORES = 8
LANES = B * D                        # 4096 independent (b, d) lanes
LANES_PER_CORE = LANES // N_CORES    # 512
P = 128                              # SBUF partitions
TILES_PER_CORE = LANES_PER_CORE // P # 4

TRACE = False       # test harness sets True to capture a neuron-profile trace
_result_info = {}   # exec_time_ns / trace path from the last run


def _build() -> bacc.Bacc:
    nc = bacc.Bacc()
    g = nc.dram_tensor(
        "gates", [LANES_PER_CORE, N], mybir.dt.float32, kind="ExternalInput"
    )
    x = nc.dram_tensor(
        "inputs", [LANES_PER_CORE, N], mybir.dt.float32, kind="ExternalInput"
    )
    o = nc.dram_tensor(
        "out", [LANES_PER_CORE, N], mybir.dt.float32, kind="ExternalOutput"
    )
    T = TILES_PER_CORE
    # Row i*P + p lands on partition p, chunk i: one contiguous 8 MiB DMA
    # per input (~97% DMA efficiency), and fresh slots mean no release waits.
    gv = g.rearrange("(i p) n -> p i n", p=P)
    xv = x.rearrange("(i p) n -> p i n", p=P)
    with TileContext(nc) as tc:
        with tc.tile_pool(name="inp", bufs=1) as pin:
            gt = pin.tile([P, T, N], mybir.dt.float32)
            xt = pin.tile([P, T, N], mybir.dt.float32)
            nc.sync.dma_start(out=gt[:, :, :], in_=gv)
            nc.sync.dma_start(out=xt[:, :, :], in_=xv)
            for i in range(T):
                # In-place: the scan writes over its inputs chunk (the DVE
                # write trails the read by the pipeline depth).
                nc.vector.tensor_tensor_scan(
                    xt[:, i, :],
                    gt[:, i, :],
                    xt[:, i, :],
                    0.0,
                    mybir.AluOpType.mult,
                    mybir.AluOpType.add,
                )
                nc.sync.dma_start(
                    out=o[i * P : (i + 1) * P, :], in_=xt[:, i, :]
                )
    nc.compile()
    return nc


def kernel(gates: np.ndarray, inputs: np.ndarray) -> np.ndarray:
    gates = np.asarray(gates, dtype=np.float32)
    inputs = np.asarray(inputs, dtype=np.float32)

    # Host-side shard: (B, N, D) -> lane-major (B*D, N); row b*D + d is the
    # contiguous time series of lane (b, d).
    gt = np.ascontiguousarray(gates.transpose(0, 2, 1)).reshape(LANES, N)
    xt = np.ascontiguousarray(inputs.transpose(0, 2, 1)).reshape(LANES, N)

    in_maps = []
    for c in range(N_CORES):
        rows = slice(c * LANES_PER_CORE, (c + 1) * LANES_PER_CORE)
        in_maps.append({"gates": gt[rows], "inputs": xt[rows]})

    nc = _build()
    res = run_bass_kernel_spmd(
        nc, in_maps, core_ids=list(range(N_CORES)), trace=TRACE
    )
    _result_info["exec_time_ns"] = res.exec_time_ns
    _result_info["mean_exec_time_ns"] = res.mean_exec_time_ns
    _result_info["profile_json"] = res.profile_json
    _result_info["trace"] = (
        res.instructions_and_trace[1] if res.instructions_and_trace else None
    )

    out_t = np.concatenate([r["out"] for r in res.results], axis=0)  # (LANES, N)
    return np.ascontiguousarray(out_t.reshape(B, D, N).transpose(0, 2, 1))


# revision 9
# speedup vs baseline: 1.2150x; 1.2150x over previous
"""AssocScan Trainium2 kernel: out[:, t] = gates[:, t] * out[:, t-1] + inputs[:, t].

Strategy: the recurrence is independent per (b, d) lane (B*D = 4096 lanes,
N = 4096 steps). The DVE `tensor_tensor_scan` instruction computes exactly
this recurrence along the free dimension, one lane per partition.

Sharding: lanes are split evenly across the 8 NeuronCores (512 lanes each).
During host-side sharding the (B, N, D) inputs are transposed to lane-major
(B*D, N) so every device DMA is fully contiguous (time series per lane
contiguous in DRAM); each core loads its shard with two 8 MiB DMAs, scans
4 tiles of [128 lanes, 4096 steps] in place on the VectorEngine, and
streams the results back.
"""

import sys

import numpy as np

for _p in ("/opt/trn_rl_repo", "/opt/pypackages"):
    if _p not in sys.path:
        sys.path.append(_p)

import concourse.bacc as bacc
import concourse.mybir as mybir
from concourse.bass_utils import run_bass_kernel_spmd
from concourse.tile import TileContext

B, N, D = 4, 4096, 1024
N_CORES = 8
LANES = B * D                        # 4096 independent (b, d) lanes
LANES_PER_CORE = LANES // N_CORES    # 512
P = 128                              # SBUF partitions
TILES_PER_CORE = LANES_PER_CORE // P # 4

TRACE = False       # test harness sets True to capture a neuron-profile trace
_result_info = {}   # exec_time_ns / trace path from the last run


def _build() -> bacc.Bacc:
    nc = bacc.Bacc()
    g = nc.dram_tensor(
        "gates", [LANES_PER_CORE, N], mybir.dt.float32, kind="ExternalInput"
    )
    x = nc.dram_tensor(
        "inputs", [LANES_PER_CORE, N], mybir.dt.float32, kind="ExternalInput"
    )
    o = nc.dram_tensor(
        "out", [LANES_PER_CORE, N], mybir.dt.float32, kind="ExternalOutput"
    )
    T = TILES_PER_CORE
    H = N // 2  # scan/store in N-halves so stores start before a chunk ends
    with TileContext(nc) as tc:
        with tc.tile_pool(name="pool", bufs=3) as pool:
            for i in range(T):
                rows = slice(i * P, (i + 1) * P)
                gt = pool.tile([P, N], mybir.dt.float32, tag="g")
                xt = pool.tile([P, N], mybir.dt.float32, tag="x")
                # Interleaved loads on the sync-engine HWDGE ring: FIFO
                # drain makes chunk completions sequential, so scan i only
                # waits for ~its own chunk, not the whole shard.
                nc.sync.dma_start(out=gt[:, :], in_=g[rows, :])
                nc.sync.dma_start(out=xt[:, :], in_=x[rows, :])
                # In-place scans (the DVE write trails the read by the
                # pipeline depth); second half chains via its carry.
                nc.vector.tensor_tensor_scan(
                    xt[:, :H],
                    gt[:, :H],
                    xt[:, :H],
                    0.0,
                    mybir.AluOpType.mult,
                    mybir.AluOpType.add,
                )
                # Stores ride the scalar-engine HWDGE ring so their waits
                # never stall load issue on the sync ring.
                nc.scalar.dma_start(out=o[rows, :H], in_=xt[:, :H])
                nc.vector.tensor_tensor_scan(
                    xt[:, H:],
                    gt[:, H:],
                    xt[:, H:],
                    xt[:, H - 1 : H],
                    mybir.AluOpType.mult,
                    mybir.AluOpType.add,
                )
                nc.scalar.dma_start(out=o[rows, H:], in_=xt[:, H:])
    nc.compile()
    return nc


def kernel(gates: np.ndarray, inputs: np.ndarray) -> np.ndarray:
    gates = np.asarray(gates, dtype=np.float32)
    inputs = np.asarray(inputs, dtype=np.float32)

    # Host-side shard: (B, N, D) -> lane-major (B*D, N); row b*D + d is the
    # contiguous time series of lane (b, d).
    gt = np.ascontiguousarray(gates.transpose(0, 2, 1)).reshape(LANES, N)
    xt = np.ascontiguousarray(inputs.transpose(0, 2, 1)).reshape(LANES, N)

    in_maps = []
    for c in range(N_CORES):
        rows = slice(c * LANES_PER_CORE, (c + 1) * LANES_PER_CORE)
        in_maps.append({"gates": gt[rows], "inputs": xt[rows]})

    nc = _build()
    res = run_bass_kernel_spmd(
        nc, in_maps, core_ids=list(range(N_CORES)), trace=TRACE
    )
    _result_info["exec_time_ns"] = res.exec_time_ns
    _result_info["mean_exec_time_ns"] = res.mean_exec_time_ns
    _result_info["profile_json"] = res.profile_json
    _result_info["trace"] = (
        res.instructions_and_trace[1] if res.instructions_and_trace else None
    )

    out_t = np.concatenate([r["out"] for r in res.results], axis=0)  # (LANES, N)
    return np.ascontiguousarray(out_t.reshape(B, D, N).transpose(0, 2, 1))


# revision 12
# speedup vs baseline: 1.6292x; 1.3409x over previous
"""AssocScan Trainium2 kernel: out[:, t] = gates[:, t] * out[:, t-1] + inputs[:, t].

Strategy: the recurrence is independent per (b, d) lane (B*D = 4096 lanes,
N = 4096 steps). The DVE `tensor_tensor_scan` instruction computes exactly
this recurrence along the free dimension, one lane per partition.

Sharding: lanes are split evenly across the 8 NeuronCores (512 lanes each).
During host-side sharding the (B, N, D) inputs are transposed to lane-major
(B*D, N) so every device DMA is fully contiguous (time series per lane
contiguous in DRAM); each core loads its shard with two 8 MiB DMAs, scans
4 tiles of [128 lanes, 4096 steps] in place on the VectorEngine, and
streams the results back.
"""

import sys

import numpy as np

for _p in ("/opt/trn_rl_repo", "/opt/pypackages"):
    if _p not in sys.path:
        sys.path.append(_p)

import concourse.bacc as bacc
import concourse.mybir as mybir
from concourse.bass_utils import run_bass_kernel_spmd
from concourse.tile import TileContext

B, N, D = 4, 4096, 1024
N_CORES = 8
LANES = B * D                        # 4096 independent (b, d) lanes
LANES_PER_CORE = LANES // N_CORES    # 512
P = 128                              # SBUF partitions
TILES_PER_CORE = LANES_PER_CORE // P # 4

TRACE = False       # test harness sets True to capture a neuron-profile trace
USE_BF16 = False    # bf16 input storage (halves load bytes); fp32 scan state
_result_info = {}   # exec_time_ns / trace path from the last run


def _build() -> bacc.Bacc:
    in_dt = mybir.dt.bfloat16 if USE_BF16 else mybir.dt.float32
    nc = bacc.Bacc()
    g = nc.dram_tensor("gates", [LANES_PER_CORE, N], in_dt, kind="ExternalInput")
    x = nc.dram_tensor("inputs", [LANES_PER_CORE, N], in_dt, kind="ExternalInput")
    o = nc.dram_tensor(
        "out", [LANES_PER_CORE, N], mybir.dt.float32, kind="ExternalOutput"
    )
    T = TILES_PER_CORE
    H = N // 2  # scan/store in N-halves so stores start before a chunk ends
    with TileContext(nc) as tc:
        with tc.tile_pool(name="pool", bufs=3) as pool:
            for i in range(T):
                rows = slice(i * P, (i + 1) * P)
                gt = pool.tile([P, N], in_dt, tag="g")
                xt = pool.tile([P, N], in_dt, tag="x")
                # The scan's internal state is fp32 regardless of operand
                # dtype; with a separate fp32 output tile the only bf16 loss
                # is input quantization.
                if USE_BF16:
                    ot = pool.tile([P, N], mybir.dt.float32, tag="o", name="ot")
                else:
                    ot = xt
                # Interleaved loads on the sync-engine HWDGE ring: FIFO
                # drain makes chunk completions sequential, so scan i only
                # waits for ~its own chunk, not the whole shard.
                nc.sync.dma_start(out=gt[:, :], in_=g[rows, :])
                nc.sync.dma_start(out=xt[:, :], in_=x[rows, :])
                # fp32 path scans in place (the DVE write trails the read by
                # the pipeline depth); second half chains via its carry.
                nc.vector.tensor_tensor_scan(
                    ot[:, :H],
                    gt[:, :H],
                    xt[:, :H],
                    0.0,
                    mybir.AluOpType.mult,
                    mybir.AluOpType.add,
                )
                # Stores ride the scalar-engine HWDGE ring so their waits
                # never stall load issue on the sync ring.
                nc.scalar.dma_start(out=o[rows, :H], in_=ot[:, :H])
                nc.vector.tensor_tensor_scan(
                    ot[:, H:],
                    gt[:, H:],
                    xt[:, H:],
                    ot[:, H - 1 : H],
                    mybir.AluOpType.mult,
                    mybir.AluOpType.add,
                )
                nc.scalar.dma_start(out=o[rows, H:], in_=ot[:, H:])
    nc.compile()
    return nc


def kernel(gates: np.ndarray, inputs: np.ndarray) -> np.ndarray:
    gates = np.asarray(gates, dtype=np.float32)
    inputs = np.asarray(inputs, dtype=np.float32)

    # Host-side shard: (B, N, D) -> lane-major (B*D, N); row b*D + d is the
    # contiguous time series of lane (b, d).
    gt = np.ascontiguousarray(gates.transpose(0, 2, 1)).reshape(LANES, N)
    xt = np.ascontiguousarray(inputs.transpose(0, 2, 1)).reshape(LANES, N)
    if USE_BF16:
        import ml_dtypes

        gt = gt.astype(ml_dtypes.bfloat16)
        xt = xt.astype(ml_dtypes.bfloat16)

    in_maps = []
    for c in range(N_CORES):
        rows = slice(c * LANES_PER_CORE, (c + 1) * LANES_PER_CORE)
        in_maps.append({"gates": gt[rows], "inputs": xt[rows]})

    nc = _build()
    res = run_bass_kernel_spmd(
        nc, in_maps, core_ids=list(range(N_CORES)), trace=TRACE
    )
    _result_info["exec_time_ns"] = res.exec_time_ns
    _result_info["mean_exec_time_ns"] = res.mean_exec_time_ns
    _result_info["profile_json"] = res.profile_json
    _result_info["trace"] = (
        res.instructions_and_trace[1] if res.instructions_and_trace else None
    )

    out_t = np.concatenate([r["out"] for r in res.results], axis=0)  # (LANES, N)
    return np.ascontiguousarray(out_t.reshape(B, D, N).transpose(0, 2, 1))
